# revision 7
# baseline (speedup 1.0000x reference)
"""Trainium2 Bass kernel for nn_CaptionHead (segment_reduce).

Computes, for full-size inputs:
    point_feats = adapter_feats[v2p_map]            # [N_PTS, D]
    gathered    = point_feats[point_idx]            # [T, D]
    sums        = segment_sum(gathered, seg_ids, S) # [S, D]
    pooled      = l2norm(sums / max(counts, 1))     # == l2norm(sums)
    logits      = (pooled @ l2norm(ce).T) * exp(logit_scale)

Distribution: adapter_feats is sharded by voxel across the 8 cores
(25000 rows each, so shard-local indices fit the int16 dma_gather path).
Each point is routed (host-side index preprocessing) to the core owning
its voxel, grouped by 128-segment chunk, and padded so every
(core, chunk) cell has the same tile count -> one SPMD program.

On device, each core gathers its points' rows with dma_gather, reduces
them into per-chunk [128, 256] PSUM accumulators with one-hot matmuls
(onehot[k, m] = (seg_local[k] == m)), ReduceScatters the [S, D] partial
sums so core r ends up with segment rows [r*256, (r+1)*256), normalizes
(the 1/count factor cancels inside l2norm), and multiplies against the
caption embeddings with the row/col norm factors applied around the
matmul.  Core r returns logits rows [r*256, (r+1)*256); the host
concatenates the blocks.
"""

import math

import numpy as np

N_VOX = 200000
N_PTS = 500000
T_FULL = 1000000
S_FULL = 2048
D_FULL = 256
N_CORES = 8
P = 128


def _preprocess(v2p_map, point_idx, seg_ids, n_cores, vox_per_core, n_chunks):
    """Route points to voxel-owning cores, group by segment chunk, pad.

    Returns (idx16, segf, tiles_per_chunk):
      idx16[m]: [128, NIDX//16] int16 shard-local voxel index per point in
                dma_gather's 16-partition-wrapped, 8x-replicated layout.
      segf[m]:  [128, NT] float32 chunk-local segment id per point
                (tile t, partition p -> point t*128 + p); -1 for padding.
    """
    v2p = np.asarray(v2p_map).astype(np.int64)
    pidx = np.asarray(point_idx).astype(np.int64)
    seg = np.asarray(seg_ids).astype(np.int64)
    cidx = v2p[pidx]                      # composed voxel index per point
    core = cidx // vox_per_core
    chunk = seg >> 7                      # 128 segments per chunk
    key = (core * n_chunks + chunk).astype(np.int32)
    order = np.argsort(key, kind="stable")
    cidx_s = cidx[order]
    seg_s = seg[order]
    counts = np.bincount(key, minlength=n_cores * n_chunks)
    tiles_per_chunk = max(1, math.ceil(counts.max() / P))
    npc = tiles_per_chunk * P
    lvox = np.zeros((n_cores, n_chunks, npc), np.int16)
    segl = np.full((n_cores, n_chunks, npc), -1.0, np.float32)
    offs = np.concatenate([[0], np.cumsum(counts)])
    for m in range(n_cores):
        for c in range(n_chunks):
            k = m * n_chunks + c
            a, b = offs[k], offs[k + 1]
            n = b - a
            lvox[m, c, :n] = (cidx_s[a:b] - m * vox_per_core).astype(np.int16)
            segl[m, c, :n] = (seg_s[a:b] - c * P).astype(np.float32)

    idx16 = []
    segf = []
    for m in range(n_cores):
        arr = lvox[m].reshape(-1, 16).T          # [16, NIDX//16]
        idx16.append(np.ascontiguousarray(np.tile(arr, (8, 1))))
        segf.append(np.ascontiguousarray(segl[m].reshape(-1, P).T))
    return idx16, segf, tiles_per_chunk


def _build_nc(tiles_per_chunk, vox_per_core, D, S, n_cores, batch_tiles=8,
              main_reps=1, mode="full"):
    """mode: "full" | "nomm" (gathers only) | "nogather" (compute only)
    | "noonehot" (gather + matmul, constant weights).  main_reps repeats the
    main loop; with mode="full" the output stays correct (each rep recomputes
    the same sums; only the last is copied out)."""
    import concourse.bacc as bacc
    import concourse.mybir as mybir
    import concourse.tile as tile
    from concourse.masks import make_identity

    f32 = mybir.dt.float32
    i16 = mybir.dt.int16
    n_chunks = S // P
    NT = n_chunks * tiles_per_chunk            # total point tiles
    NIDX = NT * P                              # total gathered rows
    out_rows = S // n_cores                    # 256
    blk_tiles = out_rows // P                  # 2
    k_tiles = D // P                           # 2
    n_cols = 512                               # moving-operand tile width
    n_tiles_out = S // n_cols                  # 4

    nc = bacc.Bacc(
        "TRN2",
        target_bir_lowering=False,
        debug=False,
        enable_asserts=False,
        num_devices=n_cores,
        # SWDGE descriptor-ring carveout: must hold two in-flight
        # dma_gathers of batch_tiles*128 descriptors each.
        dynamic_dma_scratch_size=32768,
    )

    adapter = nc.dram_tensor("adapter", [vox_per_core, D], f32, kind="ExternalInput")
    idx16 = nc.dram_tensor("idx16", [P, NIDX // 16], i16, kind="ExternalInput")
    segf = nc.dram_tensor("segf", [P, NT], f32, kind="ExternalInput")
    iota = nc.dram_tensor("iota", [P, P], f32, kind="ExternalInput")
    ce = nc.dram_tensor("ce", [S, D], f32, kind="ExternalInput")
    cet = nc.dram_tensor("cet", [D, S], f32, kind="ExternalInput")
    lsr = nc.dram_tensor("lsr", [P, 1], f32, kind="ExternalInput")
    out = nc.dram_tensor("logits_block", [out_rows, S], f32, kind="ExternalOutput")
    cc_in = nc.dram_tensor("cc_in", [S, D], f32, kind="Internal")
    cc_out = nc.dram_tensor("cc_out", [out_rows, D], f32, kind="Internal")

    with tile.TileContext(nc) as tc:
        with (
            tc.tile_pool(name="const", bufs=1) as constp,
            tc.tile_pool(name="gather", bufs=3) as gp,
            tc.tile_pool(name="oh", bufs=4) as ohp,
            tc.tile_pool(name="misc", bufs=1) as miscp,
        ):
            idx_sb = constp.tile([P, NIDX // 16], i16)
            nc.sync.dma_start(idx_sb[:], idx16.ap())
            segf_sb = constp.tile([P, NT], f32)
            nc.sync.dma_start(segf_sb[:], segf.ap())
            iota_sb = constp.tile([P, P], f32)
            nc.sync.dma_start(iota_sb[:], iota.ap())
            ident = constp.tile([P, P], f32)
            make_identity(nc, ident[:])

            sums_sb = miscp.tile([P, n_chunks * D], f32)

            # ---- main: gather + one-hot matmul segment reduction ----
            g_static = None
            if mode == "nogather":
                g_static = miscp.tile([P, batch_tiles, D], f32)
                nc.vector.memset(g_static[:], 1.0)
            if mode == "nomm":
                nc.vector.memset(sums_sb[:], 1.0)
            with tc.tile_pool(name="acc", bufs=8, space="PSUM") as accp:
                for rep in range(main_reps):
                    for c in range(n_chunks):
                        acc = None
                        if mode != "nomm":
                            acc = accp.tile([P, D], f32, tag="acc", name="acc")
                        done = 0
                        while done < tiles_per_chunk:
                            bt = min(batch_tiles, tiles_per_chunk - done)
                            if mode == "nogather":
                                g = g_static
                            else:
                                g = gp.tile([P, batch_tiles, D], f32, tag="g",
                                            name="g")
                                col0 = (c * tiles_per_chunk + done) * P // 16
                                nidx = bt * P
                                nc.gpsimd.dma_gather(
                                    out_ap=g[:, :bt, :],
                                    in_ap=adapter.ap(),
                                    idxs_ap=idx_sb[:, col0 : col0 + nidx // 16],
                                    num_idxs=nidx,
                                    num_idxs_reg=nidx,
                                    elem_size=D,
                                )
                            if mode == "nomm":
                                done += bt
                                continue
                            for jj in range(bt):
                                tglob = c * tiles_per_chunk + done + jj
                                if mode == "noonehot":
                                    oh = ident
                                else:
                                    oh = ohp.tile([P, P], f32, tag="oh", name="oh")
                                    nc.vector.tensor_tensor(
                                        out=oh[:],
                                        in0=segf_sb[:, tglob : tglob + 1].to_broadcast([P, P]),
                                        in1=iota_sb[:],
                                        op=mybir.AluOpType.is_equal,
                                    )
                                nc.tensor.matmul(
                                    acc[:],
                                    lhsT=oh[:],
                                    rhs=g[:, jj, :],
                                    start=(done + jj == 0),
                                    stop=(done + jj == tiles_per_chunk - 1),
                                )
                            done += bt
                        if mode != "nomm" and rep == main_reps - 1:
                            nc.vector.tensor_copy(
                                out=sums_sb[:, c * D : (c + 1) * D], in_=acc[:]
                            )

            # ---- all-reduce partial sums: core r keeps segment rows r*256.. ----
            cc_in_ap = cc_in.ap().rearrange("(c p) d -> p c d", p=P)
            nc.sync.dma_start(
                cc_in_ap, sums_sb[:].rearrange("p (c d) -> p c d", d=D)
            )
            nc.gpsimd.collective_compute(
                "ReduceScatter",
                mybir.AluOpType.add,
                replica_groups=[list(range(n_cores))],
                ins=[cc_in.ap()],
                outs=[cc_out.ap()],
            )

            # ---- finale: normalize + logits block ----
            with (
                tc.tile_pool(name="fin", bufs=1) as finp,
                tc.tile_pool(name="fpsum", bufs=2, space="PSUM") as fpp,
                tc.tile_pool(name="cestream", bufs=2) as cep,
            ):
                # exp(logit_scale), replicated per partition
                ls_sb = finp.tile([P, 1], f32)
                nc.sync.dma_start(ls_sb[:], lsr.ap())
                els = finp.tile([P, 1], f32)
                nc.scalar.activation(
                    els[:], ls_sb[:], mybir.ActivationFunctionType.Exp
                )

                # caption-embedding column scales: 1/max(||ce_n||, 1e-12)
                css = finp.tile([P, S // P], f32)
                sq_scr = finp.tile([P, D], f32)
                for t in range(S // P):
                    cetile = cep.tile([P, D], f32, tag="ce")
                    nc.sync.dma_start(cetile[:], ce.ap()[t * P : (t + 1) * P, :])
                    nc.scalar.activation(
                        sq_scr[:],
                        cetile[:],
                        mybir.ActivationFunctionType.Square,
                        accum_out=css[:, t : t + 1],
                    )
                csn = finp.tile([P, S // P], f32)
                nc.scalar.sqrt(csn[:], css[:])
                nc.vector.tensor_scalar_max(csn[:], csn[:], 1e-12)
                csi = finp.tile([P, S // P], f32)
                nc.vector.reciprocal(csi[:], csn[:])
                # transpose [128, 16] -> [16, 128], flatten to [1, S]
                cst_ps = fpp.tile([S // P, P], f32, tag="tps", bufs=2)
                nc.tensor.transpose(cst_ps[:], csi[:], ident[:])
                cst_sb = finp.tile([S // P, P], f32)
                nc.vector.tensor_copy(out=cst_sb[:], in_=cst_ps[:])
                csrow = finp.tile([1, S], f32)
                nc.sync.dma_start(
                    csrow[:].rearrange("a (c f) -> a c f", f=P), cst_sb[:]
                )
                # broadcast col scales across partitions via K=1 matmul
                ones_row = finp.tile([1, P], f32)
                nc.vector.memset(ones_row[:], 1.0)
                colbc = finp.tile([P, S], f32)
                for n in range(n_tiles_out):
                    cb_ps = fpp.tile([P, n_cols], f32, tag="cb", bufs=2)
                    nc.tensor.matmul(
                        cb_ps[:],
                        lhsT=ones_row[:],
                        rhs=csrow[:, n * n_cols : (n + 1) * n_cols],
                        start=True,
                        stop=True,
                    )
                    nc.vector.tensor_copy(
                        out=colbc[:, n * n_cols : (n + 1) * n_cols], in_=cb_ps[:]
                    )

                # pooled rows: load RS result, scale by exp(ls)/||row||
                rs_inv = finp.tile([P, blk_tiles], f32)
                pn = []
                blks = []
                for m in range(blk_tiles):
                    blk = finp.tile([P, D], f32, tag=f"blk{m}", name=f"blk{m}")
                    nc.sync.dma_start(
                        blk[:], cc_out.ap()[m * P : (m + 1) * P, :]
                    )
                    blks.append(blk)
                    nc.scalar.activation(
                        sq_scr[:],
                        blk[:],
                        mybir.ActivationFunctionType.Square,
                        accum_out=rs_inv[:, m : m + 1],
                    )
                rs_n = finp.tile([P, blk_tiles], f32)
                nc.scalar.sqrt(rs_n[:], rs_inv[:])
                nc.vector.tensor_scalar_max(rs_n[:], rs_n[:], 1e-12)
                rs_r = finp.tile([P, blk_tiles], f32)
                nc.vector.reciprocal(rs_r[:], rs_n[:])
                rs_s = finp.tile([P, blk_tiles], f32)
                nc.vector.tensor_tensor(
                    out=rs_s[:],
                    in0=rs_r[:],
                    in1=els[:].to_broadcast([P, blk_tiles]),
                    op=mybir.AluOpType.mult,
                )
                for m in range(blk_tiles):
                    nc.vector.tensor_tensor(
                        out=blks[m][:],
                        in0=blks[m][:],
                        in1=rs_s[:, m : m + 1].to_broadcast([P, D]),
                        op=mybir.AluOpType.mult,
                    )

                # transpose pooled block: pT[k][:, m*128..] = blk[m][:, k*128..].T
                pT = [finp.tile([P, out_rows], f32, tag=f"pT{k}", name=f"pT{k}") for k in range(k_tiles)]
                for k in range(k_tiles):
                    for m in range(blk_tiles):
                        t_ps = fpp.tile([P, P], f32, tag="tps", bufs=2)
                        nc.tensor.transpose(
                            t_ps[:], blks[m][:, k * P : (k + 1) * P], ident[:]
                        )
                        nc.vector.tensor_copy(
                            out=pT[k][:, m * P : (m + 1) * P], in_=t_ps[:]
                        )

                # load transposed caption embeds
                cet_sb = [finp.tile([P, S], f32, tag=f"cet{k}", name=f"cet{k}") for k in range(k_tiles)]
                for k in range(k_tiles):
                    nc.sync.dma_start(
                        cet_sb[k][:], cet.ap()[k * P : (k + 1) * P, :]
                    )

                # logits block: out[m*128+r, n] with column scales fused
                out_sb = [finp.tile([P, S], f32, tag=f"os{m}", name=f"os{m}") for m in range(blk_tiles)]
                for m in range(blk_tiles):
                    for n in range(n_tiles_out):
                        o_ps = fpp.tile([P, n_cols], f32, tag="ops", bufs=2)
                        for k in range(k_tiles):
                            nc.tensor.matmul(
                                o_ps[:],
                                lhsT=pT[k][:, m * P : (m + 1) * P],
                                rhs=cet_sb[k][:, n * n_cols : (n + 1) * n_cols],
                                start=(k == 0),
                                stop=(k == k_tiles - 1),
                            )
                        nc.vector.tensor_tensor(
                            out=out_sb[m][:, n * n_cols : (n + 1) * n_cols],
                            in0=o_ps[:],
                            in1=colbc[:, n * n_cols : (n + 1) * n_cols],
                            op=mybir.AluOpType.mult,
                        )
                    nc.sync.dma_start(
                        out.ap()[m * P : (m + 1) * P, :], out_sb[m][:]
                    )
    nc.compile()
    return nc


def _make_in_maps(adapter_feats, caption_embed, logit_scale, idx16, segf,
                  n_cores, vox_per_core):
    af = np.ascontiguousarray(np.asarray(adapter_feats, np.float32))
    ce_np = np.ascontiguousarray(np.asarray(caption_embed, np.float32))
    cet_np = np.ascontiguousarray(ce_np.T)
    ls = np.asarray(logit_scale, np.float32).reshape(-1)[0]
    ls_rep = np.full((P, 1), ls, np.float32)
    iota_mat = np.ascontiguousarray(
        np.broadcast_to(np.arange(P, dtype=np.float32), (P, P))
    )
    in_maps = []
    for m in range(n_cores):
        in_maps.append(
            {
                "adapter": af[m * vox_per_core : (m + 1) * vox_per_core],
                "idx16": idx16[m],
                "segf": segf[m],
                "iota": iota_mat,
                "ce": ce_np,
                "cet": cet_np,
                "lsr": ls_rep,
            }
        )
    return in_maps


def _run(inputs_dict, n_cores, vox_per_core, D, S, batch_tiles=8, trace=False):
    from concourse.bass_utils import run_bass_kernel_spmd

    idx16, segf, tiles_per_chunk = _preprocess(
        inputs_dict["v2p_map"],
        inputs_dict["point_idx"],
        inputs_dict["seg_ids"],
        n_cores,
        vox_per_core,
        S // P,
    )
    nc = _build_nc(tiles_per_chunk, vox_per_core, D, S, n_cores, batch_tiles)
    in_maps = _make_in_maps(
        inputs_dict["adapter_feats"],
        inputs_dict["caption_embed"],
        inputs_dict["logit_scale"],
        idx16,
        segf,
        n_cores,
        vox_per_core,
    )
    res = run_bass_kernel_spmd(
        nc, in_maps, core_ids=list(range(n_cores)), trace=trace
    )
    blocks = [res.results[m]["logits_block"] for m in range(n_cores)]
    return np.concatenate(blocks, axis=0), res


def kernel(adapter_feats, caption_embed, logit_scale, v2p_map, point_idx,
           seg_ids, num_segments=S_FULL, **_):
    logits, _res = _run(
        {
            "adapter_feats": adapter_feats,
            "caption_embed": caption_embed,
            "logit_scale": logit_scale,
            "v2p_map": v2p_map,
            "point_idx": point_idx,
            "seg_ids": seg_ids,
        },
        N_CORES,
        N_VOX // N_CORES,
        D_FULL,
        S_FULL,
    )
    return logits
